# revision 20
# baseline (speedup 1.0000x reference)
"""Sliding-window (radius-8, K=17) single-head attention along W.

Full problem: feature/position [2, 128, 64, 256] f32; 1x1 convs Wq/Wk (+bias)
produce q/k; scores over a 17-wide window along W; softmax (zero-padded
windows contribute exp(0)=1 to the denominator); output is the attn-weighted
sum of windows of x = feature + position.

Sharding: data-parallel over (B, H) — the 128 (b, h) rows are independent;
each of the 8 cores gets 16 rows, two per iteration. Per row
(x_row = [C=128, W=256]):
  q = (Wq/sqrt(C)) x + bq/sqrt(C);  k = Wk x + bk        (PE matmuls + bias)
  S^T[w', w] = k^T q + bandmask^T   computed TRANSPOSED (keys on partitions)
      so that exp(S^T) lands in SBUF already in the layout the output matmul
      needs — no attention transposes. The mask is pre-written into PSUM by a
      PE copy-matmul (ident.T @ maskT); score matmuls accumulate on top.
  attU = exp(S^T) bf16                                   (unnormalized)
  den[w] (broadcast to all partitions) = ones128.T @ attU, accumulated on top
      of ident.T @ oob_bc (the zero-padded out-of-range counts, exp(0)=1
      each); normalization happens at the end: out = (x @ attU) * recip(den).
  out_u = x^T.T @ attU  (PE transposes of x, then accumulating matmuls)

The matmul path runs in bf16 (fast weight load + 1 cyc/row); scores
accumulate in fp32 PSUM and the exp input stays fp32. Softmax skips
max-subtraction: scores are O(10) here, well inside exp/fp32 range
(unnormalized attU and den stay within fp32/bf16 range too).
"""

import numpy as np
from contextlib import ExitStack

import concourse.bass as bass
import concourse.bacc as bacc
import concourse.mybir as mybir
import concourse.tile as tile
from concourse.bass_utils import run_bass_kernel_spmd

B, C, H, W = 2, 128, 64, 256
R = 8
NCORES = 8
ROWS = B * H // NCORES        # 16 (b, h) rows per core
CORES_PER_B = NCORES // B     # 4
F32 = mybir.dt.float32
BF = mybir.dt.bfloat16
EXP = mybir.ActivationFunctionType.Exp
NEG = -1e9
RL = 4                        # rows per input DMA


def build_nc():
    nc = bacc.Bacc(trn_type="TRN2")
    f_ext = nc.dram_tensor("feature", [C, ROWS, W], F32, kind="ExternalInput")
    p_ext = nc.dram_tensor("position", [C, ROWS, W], F32, kind="ExternalInput")
    wq_ext = nc.dram_tensor("wqt", [C, C], BF, kind="ExternalInput")
    wk_ext = nc.dram_tensor("wkt", [C, C], BF, kind="ExternalInput")
    id_ext = nc.dram_tensor("ident", [C, C], BF, kind="ExternalInput")
    ones_ext = nc.dram_tensor("ones", [C, C], BF, kind="ExternalInput")
    bq_ext = nc.dram_tensor("bqv", [C, 1], F32, kind="ExternalInput")
    bk_ext = nc.dram_tensor("bkv", [C, 1], F32, kind="ExternalInput")
    mask_ext = nc.dram_tensor("maskT", [C, 2 * W], BF, kind="ExternalInput")
    oob_ext = nc.dram_tensor("oob_bc", [C, 2 * W], BF, kind="ExternalInput")
    out_ext = nc.dram_tensor("out", [C, ROWS, W], F32, kind="ExternalOutput")

    with tile.TileContext(nc) as tc, ExitStack() as ctx:
        const = ctx.enter_context(tc.tile_pool(name="const", bufs=1))
        inp = ctx.enter_context(tc.tile_pool(name="inp", bufs=2))

        # first input tiles load before the constants so compute starts early;
        # split into 2-row chunks and dual-issue on the SP and ACT HWDGE rings
        ft0 = inp.tile([C, RL, W], F32, tag="ft")
        nc.sync.dma_start(ft0[:, 0:2], f_ext[:, 0:2, :])
        pt0 = inp.tile([C, RL, W], F32, tag="pt")
        nc.scalar.dma_start(pt0[:, 0:2], p_ext[:, 0:2, :])
        nc.sync.dma_start(ft0[:, 2:RL], f_ext[:, 2:RL, :])
        nc.scalar.dma_start(pt0[:, 2:RL], p_ext[:, 2:RL, :])

        def cload(shape, dt, ext, tag):
            t = const.tile(shape, dt, tag=tag)
            nc.scalar.dma_start(t[:], ext[:])
            return t

        wq_t = cload([C, C], BF, wq_ext, "wq")
        wk_t = cload([C, C], BF, wk_ext, "wk")
        ident = cload([C, C], BF, id_ext, "id")
        ones_t = cload([C, C], BF, ones_ext, "ones")
        bq_t = cload([C, 1], F32, bq_ext, "bq")
        bk_t = cload([C, 1], F32, bk_ext, "bk")
        mask_t = cload([C, 2 * W], BF, mask_ext, "mask")
        oob_t = cload([C, 2 * W], BF, oob_ext, "oob")

        # touch Exp once so the ACT table loads during the input-DMA ramp
        warm = const.tile([C, 1], F32, tag="warm")
        nc.scalar.activation(warm[:], bq_t[:], EXP)
        xp = ctx.enter_context(tc.tile_pool(name="x", bufs=4))
        qkp = ctx.enter_context(tc.tile_pool(name="qk", bufs=4))
        attp = ctx.enter_context(tc.tile_pool(name="att", bufs=4))
        sbT = ctx.enter_context(tc.tile_pool(name="sbT", bufs=4))
        rdp = ctx.enter_context(tc.tile_pool(name="rd", bufs=4))
        psqk = ctx.enter_context(tc.tile_pool(name="psqk", bufs=2, space="PSUM"))
        pss = ctx.enter_context(tc.tile_pool(name="pss", bufs=3, space="PSUM"))
        psxt = ctx.enter_context(tc.tile_pool(name="psxt", bufs=1, space="PSUM"))
        pso = ctx.enter_context(tc.tile_pool(name="pso", bufs=2, space="PSUM"))

        ft, pt = ft0, pt0
        for it in range(ROWS // 2):
            r = 2 * it
            if r % RL == 0 and r > 0:
                ft = inp.tile([C, RL, W], F32, tag="ft")
                nc.sync.dma_start(ft[:], f_ext[:, r : r + RL, :])
                pt = inp.tile([C, RL, W], F32, tag="pt")
                nc.scalar.dma_start(pt[:], p_ext[:, r : r + RL, :])
            j = r % RL

            # x per row (gpsimd runs ahead of the PE chain)
            xt = xp.tile([C, 2, W], BF)
            nc.gpsimd.tensor_add(xt[:, 0], ft[:, j, :], pt[:, j, :])
            nc.gpsimd.tensor_add(xt[:, 1], ft[:, j + 1, :], pt[:, j + 1, :])

            # q|k per row: [C, 512] fp32 PSUM (1 bank each)
            # qk_sb layout: q rows at [0 : 2W], k rows at [2W : 4W]
            qk_sb = qkp.tile([C, 4 * W], BF)
            for rr in range(2):
                qk_ps = psqk.tile([C, 2 * W], F32, tag="qk")
                nc.tensor.matmul(
                    qk_ps[:, 0:W], wq_t[:], xt[:, rr], start=True, stop=True
                )
                nc.tensor.matmul(
                    qk_ps[:, W : 2 * W], wk_t[:], xt[:, rr], start=True, stop=True
                )
                nc.scalar.add(
                    qk_sb[:, rr * W : (rr + 1) * W], qk_ps[:, 0:W], bq_t[:]
                )
                nc.vector.tensor_scalar_add(
                    qk_sb[:, 2 * W + rr * W : 2 * W + (rr + 1) * W],
                    qk_ps[:, W : 2 * W],
                    bk_t[:],
                )

            # attU^T per row, straight to SBUF: att[:, r*512:(r+1)*512] is
            # [keys-chunk (2x128 partitions) | queries 0:256 free] per row.
            att = attp.tile([C, 4 * W], BF)
            for rr in range(2):
                q0 = rr * W
                k0 = 2 * W + rr * W
                s_ps = pss.tile([C, 2 * W], F32, tag="s")
                nc.tensor.matmul(s_ps[:], ident[:], mask_t[:], start=True, stop=False)
                nc.tensor.matmul(
                    s_ps[:, 0:W],
                    qk_sb[:, k0 : k0 + 128],
                    qk_sb[:, q0 : q0 + 2 * 128],
                    start=False, stop=False,
                )
                nc.tensor.matmul(
                    s_ps[:, W : 2 * W],
                    qk_sb[:, k0 + 128 : k0 + W],
                    qk_sb[:, q0 : q0 + 2 * 128],
                    start=False, stop=True,
                )
                nc.scalar.activation(
                    att[:, rr * 2 * W : (rr + 1) * 2 * W], s_ps[:], EXP
                )

            # denominators, broadcast across partitions by the ones matmul;
            # oob counts pre-accumulated from a constant.
            den_ps = pss.tile([C, 2 * W], F32, tag="s")
            nc.tensor.matmul(den_ps[:], ident[:], oob_t[:], start=True, stop=False)
            for rr in range(2):
                a0 = rr * 2 * W
                nc.tensor.matmul(
                    den_ps[:, rr * W : (rr + 1) * W],
                    ones_t[:],
                    att[:, a0 : a0 + W],
                    start=False, stop=False,
                )
                nc.tensor.matmul(
                    den_ps[:, rr * W : (rr + 1) * W],
                    ones_t[:],
                    att[:, a0 + W : a0 + 2 * W],
                    start=False, stop=(rr == 1),
                )
            rden = rdp.tile([C, 2 * W], F32)
            nc.vector.reciprocal_approx_fast(out=rden[:], in_=den_ps[:])

            # x^T chunks for the output matmul
            xt_ps = psxt.tile([C, 2 * W], BF, tag="xt")
            for rr in range(2):
                nc.tensor.transpose(
                    xt_ps[:, rr * W : rr * W + 128], xt[:, rr, 0:128], ident[:]
                )
                nc.tensor.transpose(
                    xt_ps[:, rr * W + 128 : (rr + 1) * W], xt[:, rr, 128:256], ident[:]
                )
            xT = sbT.tile([C, 2 * W], BF, tag="xT")
            nc.vector.tensor_copy(xT[:], xt_ps[:])

            o_ps = pso.tile([C, 2 * W], F32, tag="out")
            for rr in range(2):
                os_ = o_ps[:, rr * W : (rr + 1) * W]
                a0 = rr * 2 * W
                nc.tensor.matmul(
                    os_,
                    xT[:, rr * W : rr * W + 128],
                    att[:, a0 : a0 + W],
                    start=True, stop=False,
                )
                nc.tensor.matmul(
                    os_,
                    xT[:, rr * W + 128 : (rr + 1) * W],
                    att[:, a0 + W : a0 + 2 * W],
                    start=False, stop=True,
                )
            o_sb = sbT.tile([C, 2 * W], F32, tag="osb")
            nc.vector.tensor_mul(o_sb[:], o_ps[:], rden[:])
            nc.sync.dma_start(out_ext[:, r : r + 2, :], o_sb[:])

    nc.compile()
    return nc


def host_consts(Wq, bq, Wk, bk):
    import ml_dtypes

    sc = 1.0 / np.sqrt(np.float32(C))
    wqt = np.ascontiguousarray(Wq.astype(np.float32).T * sc).astype(ml_dtypes.bfloat16)
    bqv = np.ascontiguousarray((bq.astype(np.float32) * sc).reshape(C, 1))
    wkt = np.ascontiguousarray(Wk.astype(np.float32).T).astype(ml_dtypes.bfloat16)
    bkv = np.ascontiguousarray(bk.astype(np.float32).reshape(C, 1))

    ident = np.eye(C, dtype=np.float32).astype(ml_dtypes.bfloat16)
    ones = np.ones((C, C), dtype=np.float32).astype(ml_dtypes.bfloat16)

    # maskT[p, c*W + w] for key chunk c: key w' = c*128 + p, query w
    wgrid = np.arange(W)
    maskT = np.full((C, 2 * W), NEG, dtype=np.float32)
    for cchunk in range(2):
        for p in range(C):
            wk_ = cchunk * 128 + p
            lo, hi = max(0, wk_ - R), min(W, wk_ + R + 1)
            maskT[p, cchunk * W + lo : cchunk * W + hi] = 0.0
    maskT = maskT.astype(ml_dtypes.bfloat16)

    # oob count per query w, same row repeated on all partitions, two rows
    oob_row = np.maximum(0, R - wgrid) + np.maximum(0, wgrid - (W - 1 - R))
    oob_bc = np.tile(oob_row.astype(np.float32), (C, 2)).astype(ml_dtypes.bfloat16)
    return wqt, bqv, wkt, bkv, maskT, oob_bc, ident, ones


def core_inputs(feature, position, Wq, bq, Wk, bk):
    wqt, bqv, wkt, bkv, maskT, oob_bc, ident, ones = host_consts(Wq, bq, Wk, bk)
    in_maps = []
    for i in range(NCORES):
        b = i // CORES_PER_B
        h0 = (i % CORES_PER_B) * ROWS
        in_maps.append(
            {
                "feature": np.ascontiguousarray(
                    feature[b, :, h0 : h0 + ROWS, :], dtype=np.float32
                ),
                "position": np.ascontiguousarray(
                    position[b, :, h0 : h0 + ROWS, :], dtype=np.float32
                ),
                "wqt": wqt,
                "ident": ident,
                "ones": ones,
                "wkt": wkt,
                "bqv": bqv,
                "bkv": bkv,
                "maskT": maskT,
                "oob_bc": oob_bc,
            }
        )
    return in_maps


def kernel(feature, position, Wq, bq, Wk, bk):
    in_maps = core_inputs(feature, position, Wq, bq, Wk, bk)
    nc = build_nc()
    res = run_bass_kernel_spmd(nc, in_maps, list(range(NCORES)))
    out = np.empty((B, C, H, W), dtype=np.float32)
    for i in range(NCORES):
        b = i // CORES_PER_B
        h0 = (i % CORES_PER_B) * ROWS
        out[b, :, h0 : h0 + ROWS, :] = res.results[i]["out"]
    return out
